# revision 1
# baseline (speedup 1.0000x reference)
"""Trainium2 Bass kernel: batched bilinear form  out[n] = elg[n] @ W @ eth[n].

Problem: elg, eth [32768, 1024] fp32, W [1024, 1024] fp32.
Sharding: data-parallel over the batch (N) axis across 8 NeuronCores;
W is replicated.  Per core (4096 rows):

    T      = elg @ W                   (TensorE, fp16 in, fp32 PSUM accum)
    out[n] = sum_e T[n,e] * eth[n,e]   (VectorE fused multiply-reduce, fp32)

elg and W are cast to fp16 on the host (values are ~N(0,1); quantization
error is ~3e-4 of the output absmax after accumulation — PSUM accumulation
itself is fp32).  eth stays fp32 and the reduction is fp32.

The matmul contracts over the partition axis, so elg must be laid out
[d, n].  That transpose is done ON THE HOST (free; numpy) so the device
DMAs are plain contiguous loads — the DMA xbar-transpose path costs
~60 us of PE stall per core at this size.

Each 128-row output tile takes 16 matmuls (8 k-tiles x 2 psum half-banks
of 512 fp32) followed by one fused affine_mul_reduce against eth.  The
compiler splits every matmul into Ldweights+Matmult; the second matmul of
each half-bank pair reloads an identical stationary, so a post-compile
pass deletes those redundant Ldweights (the PE array already holds the
weights; the deleted loads carry no semaphore waits/updates).
"""

import numpy as np

N_TOTAL = 32768
D = 1024
N_CORES = 8
N_CORE = N_TOTAL // N_CORES          # 4096 rows per core
P = 128                              # SBUF/PSUM partitions
K_TILES = D // P                     # 8 contraction tiles
CHUNK_ROWS = 1024                    # rows per DMA chunk
TILES_PER_CHUNK = CHUNK_ROWS // P    # 8
E_HALF = 512                         # fp32 free elems per PSUM bank

_CACHE = {}


def _dedup_ldweights(nc):
    """Delete InstLdweights that reload the stationary already in the PE
    array (identical access pattern, no semaphore waits/updates).  Safe
    because only InstLdweights writes the PE weight planes."""
    removed = 0
    for blk in nc.m.functions[0].blocks:
        out = []
        last_sig = None
        for i in blk.instructions:
            if type(i).__name__ == "InstLdweights":
                si = i.sync_info
                clean = si is None or (len(si.on_wait) == 0 and len(si.on_update) == 0)
                sig = str(i.ins[0])
                if clean and sig == last_sig:
                    removed += 1
                    continue
                last_sig = sig
            out.append(i)
        blk.instructions = out
    return removed


def _sparsify_pe_sems(nc):
    """Make matmuls signal the PE tile-completion semaphore once per
    accumulation group (on the stop matmul) instead of once per matmul,
    rescaling every wait on that semaphore from matmul counts to group
    counts.  ceil() on the rescale means a wait that targeted a mid-group
    count now fires at the group boundary — later, which is conservative
    for sem-ge waits.  Aborts (returns 0) on any unexpected structure."""
    blocks = nc.m.functions[0].blocks
    pe_sems = set()
    for blk in blocks:
        for i in blk.instructions:
            if type(i).__name__ == "InstMatmult" and i.sync_info:
                for u in i.sync_info.on_update:
                    pe_sems.add(u.ant_name)
    if not pe_sems:
        return 0
    for blk in blocks:
        for i in blk.instructions:
            si = i.sync_info
            if not si:
                continue
            for u in si.on_update:
                if u.ant_name in pe_sems and (
                    type(i).__name__ != "InstMatmult" or u.update_mode != "sem-inc"
                    or u.update_value != 1
                ):
                    return 0
    # Every matmul must carry exactly one PE-sem update; record the old sem
    # count at each stop matmul.  A wait >= v is retargeted to the number of
    # stops at-or-before the first stop position >= v (exact when v lands on
    # a stop, otherwise fires at the next stop - later = conservative).
    import bisect
    stops = []
    cnt = 0
    for blk in blocks:
        for i in blk.instructions:
            if type(i).__name__ != "InstMatmult":
                continue
            ups = [u for u in (i.sync_info.on_update if i.sync_info else [])
                   if u.ant_name in pe_sems]
            if len(ups) != 1:
                return 0
            cnt += 1
            if i.stop_tensor_calc:
                stops.append(cnt)
    if not stops or stops[-1] != cnt:
        return 0
    removed = 0
    for blk in blocks:
        for i in blk.instructions:
            si = i.sync_info
            if not si:
                continue
            for w in si.on_wait:
                if w.ant_name in pe_sems:
                    w.wait_value = bisect.bisect_left(stops, w.wait_value) + 1
            if type(i).__name__ == "InstMatmult" and not i.stop_tensor_calc:
                keep = [u for u in si.on_update if u.ant_name not in pe_sems]
                if len(keep) != len(si.on_update):
                    si.on_update = keep
                    removed += 1
    return removed


def _build_program(n_core_rows, repeats=1, chunk_rows=None, lg_bufs=2, et_bufs=2,
                   ps_bufs=3, dedup=True, sparse_sems=False):
    """Build the per-core Bass program."""
    import concourse.tile as tile
    from concourse import bacc, mybir

    f16 = mybir.dt.float16
    f32 = mybir.dt.float32

    if chunk_rows is None:
        chunk_rows = CHUNK_ROWS
    assert n_core_rows % chunk_rows == 0 and chunk_rows % P == 0
    n_chunks = n_core_rows // chunk_rows
    tiles_per_chunk = chunk_rows // P
    n_tiles = n_core_rows // P

    nc = bacc.Bacc("TRN2", target_bir_lowering=False, debug=False)
    elgT16 = nc.dram_tensor("elgT16", [D, n_core_rows], f16, kind="ExternalInput").ap()
    eth = nc.dram_tensor("eth", [n_core_rows, D], f32, kind="ExternalInput").ap()
    w16 = nc.dram_tensor("w16", [D, D], f16, kind="ExternalInput").ap()
    out = nc.dram_tensor("out", [P, n_tiles * repeats], f32, kind="ExternalOutput").ap()

    with tile.TileContext(nc) as tc:
        with tc.tile_pool(name="w_pool", bufs=1) as w_pool, \
             tc.tile_pool(name="lg_pool", bufs=lg_bufs) as lg_pool, \
             tc.tile_pool(name="et_pool", bufs=et_bufs) as et_pool, \
             tc.tile_pool(name="pr_pool", bufs=2) as pr_pool, \
             tc.tile_pool(name="acc_pool", bufs=1) as acc_pool, \
             tc.tile_pool(name="ps_pool", bufs=ps_bufs, space="PSUM") as ps_pool:

            w_sb = w_pool.tile([P, K_TILES, D], f16, name="w_sb")
            for k in range(K_TILES):
                nc.sync.dma_start(
                    out=w_sb[:, k, :],
                    in_=w16[k * P:(k + 1) * P, :],
                )

            out_sb = acc_pool.tile([P, n_tiles * repeats], f32, name="out_sb")

            for _rep in range(repeats):
                r0 = 0
                t_idx = _rep * n_tiles
                for _c in range(n_chunks):
                    chunk = chunk_rows
                    elgT = lg_pool.tile([P, K_TILES, chunk_rows], f16, name="elgT")
                    for k in range(K_TILES):
                        nc.sync.dma_start(
                            out=elgT[:, k, :],
                            in_=elgT16[k * P:(k + 1) * P, r0:r0 + chunk],
                        )
                    eth_sb = et_pool.tile([P, tiles_per_chunk, D], f32, name="eth_sb")
                    for s in range(tiles_per_chunk):
                        nc.sync.dma_start(
                            out=eth_sb[:, s, :],
                            in_=eth[r0 + s * P:r0 + (s + 1) * P, :],
                        )

                    for s in range(tiles_per_chunk):
                        t_ps = ps_pool.tile([P, D], f32, name="t_ps")
                        for k in range(K_TILES):
                            for eh in range(2):
                                nc.tensor.matmul(
                                    t_ps[:, eh * E_HALF:(eh + 1) * E_HALF],
                                    elgT[:, k, s * P:(s + 1) * P],
                                    w_sb[:, k, eh * E_HALF:(eh + 1) * E_HALF],
                                    start=(k == 0),
                                    stop=(k == K_TILES - 1),
                                )
                        prod = pr_pool.tile([P, D], f32, name="prod")
                        nc.vector.affine_mul_reduce(
                            out=prod[:],
                            accum_out=out_sb[:, t_idx:t_idx + 1],
                            in0=t_ps[:],
                            in1=eth_sb[:, s, :],
                            scale=1.0,
                            bias=0.0,
                        )
                        t_idx += 1
                    r0 += chunk

            nc.sync.dma_start(out=out, in_=out_sb[:])

    nc.compile()
    if dedup:
        _dedup_ldweights(nc)
    if sparse_sems:
        _sparsify_pe_sems(nc)
    return nc


def _make_runner(nc, n_cores):
    """Mirror bass2jax.run_bass_via_pjrt's multi-core branch, but return a
    cached jitted callable so repeat calls skip retracing.
    """
    import jax
    import concourse.mybir as mybir
    from concourse import bass2jax
    from jax.experimental.shard_map import shard_map
    from jax.sharding import Mesh, PartitionSpec

    bass2jax.install_neuronx_cc_hook()
    assert nc.dbg_addr is None
    partition_name = nc.partition_id_tensor.name if nc.partition_id_tensor else None

    in_names, out_names, out_avals = [], [], []
    for alloc in nc.m.functions[0].allocations:
        if not isinstance(alloc, mybir.MemoryLocationSet):
            continue
        name = alloc.memorylocations[0].name
        if alloc.kind == "ExternalInput":
            if name != partition_name:
                in_names.append(name)
        elif alloc.kind == "ExternalOutput":
            shape = tuple(alloc.tensor_shape)
            dtype = mybir.dt.np(alloc.dtype)
            out_names.append(name)
            out_avals.append(jax.core.ShapedArray(shape, dtype))
    n_params = len(in_names)
    n_outs = len(out_avals)
    all_in_names = in_names + out_names
    if partition_name is not None:
        all_in_names = all_in_names + [partition_name]

    def _body(*args):
        operands = list(args)
        if partition_name is not None:
            operands.append(bass2jax.partition_id_tensor())
        outs = bass2jax._bass_exec_p.bind(
            *operands,
            out_avals=tuple(out_avals),
            in_names=tuple(all_in_names),
            out_names=tuple(out_names),
            lowering_input_output_aliases=(),
            sim_require_finite=True,
            sim_require_nnan=True,
            nc=nc,
        )
        return tuple(outs)

    devices = jax.devices()[:n_cores]
    assert len(devices) == n_cores
    mesh = Mesh(np.asarray(devices), ("core",))
    spec = PartitionSpec("core")
    sharded = jax.jit(
        shard_map(
            _body,
            mesh=mesh,
            in_specs=(spec,) * (n_params + n_outs),
            out_specs=(spec,) * n_outs,
            check_rep=False,
        ),
        donate_argnums=tuple(range(n_params, n_params + n_outs)),
        keep_unused=True,
    )
    zero_out_shapes = [
        ((n_cores * av.shape[0],) + tuple(av.shape[1:]), av.dtype) for av in out_avals
    ]
    return sharded, in_names, out_names, zero_out_shapes, mesh, spec


def _get_runner():
    r = _CACHE.get("runner")
    if r is None:
        nc = _build_program(N_CORE)
        r = _CACHE["runner"] = _make_runner(nc, N_CORES)
    return r


def _global_inputs(elg, eth, weight):
    """Host-side marshalling: cast + transpose + per-core-tile."""
    elg16 = elg.astype(np.float16)
    elgT16 = np.ascontiguousarray(
        elg16.reshape(N_CORES, N_CORE, D).transpose(0, 2, 1)
    ).reshape(N_CORES * D, N_CORE)
    w16 = np.broadcast_to(weight.astype(np.float16), (N_CORES, D, D)).reshape(
        N_CORES * D, D
    )
    return {"elgT16": elgT16, "eth": eth, "w16": w16}


def _call_runner(global_ins):
    sharded, in_names, out_names, zero_out_shapes, _, _ = _get_runner()
    zeros = [np.zeros(shape, dt) for shape, dt in zero_out_shapes]
    out_arrs = sharded(*[global_ins[n] for n in in_names], *zeros)
    out_g = np.asarray(out_arrs[out_names.index("out")])  # [8*128, 32]
    return np.concatenate(
        [out_g[c * P:(c + 1) * P].T.reshape(-1) for c in range(N_CORES)]
    ).astype(np.float32)


def kernel(elg, eth, weight):
    elg = np.asarray(elg, dtype=np.float32)
    eth = np.asarray(eth, dtype=np.float32)
    weight = np.asarray(weight, dtype=np.float32)
    return _call_runner(_global_inputs(elg, eth, weight))



# revision 5
# speedup vs baseline: 1.0083x; 1.0083x over previous
"""Trainium2 Bass kernel: batched bilinear form  out[n] = elg[n] @ W @ eth[n].

Problem: elg, eth [32768, 1024] fp32, W [1024, 1024] fp32.  Data-parallel
over the batch axis across 8 NeuronCores; W replicated.

Strassen (one level, 2x2x2) on the per-core block matmul: PE work drops to
7/8 of the fp16 roofline (95.6us vs 109.2us per core).  Seven half-size
products M1..M7 replace the eight naive block-matmuls.  All A/B-side block
combinations are precomputed on the host (free); the C-side combinations
are folded into the final eth-weighted reduce, whose weight tensors (eth
quadrant blocks and differences) are also host-precomputed.

Per core (4096 rows): rows split into halves n1/n2 (2048 each), d and e
split at 512.  For each 128-row "pair group" g (same row index in n1 and
n2) the device computes Mi = S_i @ T_i ([128, 512] each, contraction 512 =
4 fp16 k-tile matmuls accumulating in PSUM) and reduces

    out1[g] = (M1+M4+M7).g1 + M5.g2 + M3.g3     (rows n1)
    out2[g] = M2.g4 + M4.g5 + (M1+M3+M6).g6     (rows n2)

with g1=w11, g2=w12-w11, g3=w12, g4=w21-w22, g5=w21, g6=w22 (eth quadrant
blocks; the differences realise the Strassen minus signs so every reduce
is a plain multiply-accumulate).  Products are laid out in psum as 4
two-bank units [M3|M1], [M4|M7], [M6|M2], [M5|-] rotating through all 8
banks (pool bufs=4) so the vector engine reads adjacent product pairs in
single 1024-wide affine_mul_reduce ops (6 per group, partials combined by
4 tail adds per pass) while the PE fills the next unit.  Duplicated
reduce weights (g1 twice for [M4|M7], g6 twice for [M3|M1]-out2) sit in
extra SBUF slots filled by the otherwise-idle scalar engine.

DMA layouts are group-major slabs so every per-group transfer is fully
contiguous (>=6KB bursts): sub-512B runs pay a 2x DMA latency penalty and
made a first cut of this kernel DMA-bound (116us vs the final 100us in
TimelineSim; fp16 baseline sims at 109us).

Engine budget per pass (TimelineSim steady state): PE 98us, DVE 97us
(6 reduces x 16 groups, 10 half-widths of psum reads per group is the
minimum for this Strassen variant), DMA 78us, ACT 39us -> 100.3us
vs 109.1us for the naive fp16 kernel.

Note: nc.vector.tensor_tensor_reduce (the standard TTR ISA op) crashes
this hardware (NRT_EXEC_UNIT_UNRECOVERABLE) — the custom-DVE
affine_mul_reduce is used instead, which also forces the partial-plane
accumulation scheme (its accumulator always seeds at zero).
"""

import numpy as np

N_TOTAL = 32768
D = 1024
N_CORES = 8
N_CORE = N_TOTAL // N_CORES      # 4096 rows per core
P = 128
H = N_CORE // 2                  # 2048 rows per half
E2 = 512
G_PER = H // P                   # 16 pair-groups per core

# product execution order: M3, M1, M4, M7, M6, M2, M5 (0-based into M1..M7)
ORDER = [2, 0, 3, 6, 5, 1, 4]

_CACHE = {}


def _build_program_strassen(repeats=1, s_bufs=4, g_bufs=4, ps_bufs=4, pr_bufs=2,
                            prod_f32=False, act_cp=True):
    import concourse.tile as tile
    from concourse import bacc, mybir

    f16 = mybir.dt.float16
    f32 = mybir.dt.float32
    n_tiles = N_CORE // P        # 32

    nc = bacc.Bacc("TRN2", target_bir_lowering=False, debug=False)
    # group-major slabs, fully contiguous per group
    sT = nc.dram_tensor("sT16", [G_PER * P, 28 * P], f16, kind="ExternalInput").ap()
    g16 = nc.dram_tensor("g16", [G_PER * P, 6 * E2], f16, kind="ExternalInput").ap()
    bT = nc.dram_tensor("bT16", [P, 28 * E2], f16, kind="ExternalInput").ap()
    out = nc.dram_tensor("out", [P, n_tiles * repeats], f32, kind="ExternalOutput").ap()

    with tile.TileContext(nc) as tc:
        with tc.tile_pool(name="b_pool", bufs=1) as b_pool, \
             tc.tile_pool(name="s_pool", bufs=s_bufs) as s_pool, \
             tc.tile_pool(name="g_pool", bufs=g_bufs) as g_pool, \
             tc.tile_pool(name="pr_pool", bufs=pr_bufs) as pr_pool, \
             tc.tile_pool(name="sc_pool", bufs=2) as sc_pool, \
             tc.tile_pool(name="acc_pool", bufs=1) as acc_pool, \
             tc.tile_pool(name="ps_pool", bufs=ps_bufs, space="PSUM") as ps_pool:

            b_sb = b_pool.tile([P, 28 * E2], f16, name="b_sb")
            nc.sync.dma_start(out=b_sb[:], in_=bT[:, :])

            out_sb = acc_pool.tile([P, n_tiles * repeats], f32, name="out_sb")

            for rep in range(repeats):
                # 6 partial planes of 16 group-columns each; combined into
                # out_sb by 4 tail adds at the end of the repeat
                part = sc_pool.tile([P, 6 * G_PER], f32, name="part")
                for g in range(G_PER):
                    rows = slice(g * P, (g + 1) * P)
                    s_sb = s_pool.tile([P, 28 * P], f16, name="s_sb")
                    nc.sync.dma_start(out=s_sb[:], in_=sT[rows, :])
                    # sbuf slots: [g3, g1, g6, g4, g5, g2 | g1b, g1c, g6b, g6c]
                    g_sb = g_pool.tile([P, 10 * E2], f16, name="g_sb")
                    nc.sync.dma_start(out=g_sb[:, 0:6 * E2], in_=g16[rows, :])
                    for dst, src in ((6, 1), (7, 1), (8, 2), (9, 2)):
                        nc.scalar.copy(out=g_sb[:, dst * E2:(dst + 1) * E2],
                                       in_=g_sb[:, src * E2:(src + 1) * E2])

                    us = []
                    for u in range(4):
                        ps = ps_pool.tile([P, 1024], f32, name="ps")
                        us.append(ps)
                        for sl in range(2 if u < 3 else 1):
                            j = 2 * u + sl
                            for k in range(4):
                                nc.tensor.matmul(
                                    ps[:, sl * E2:(sl + 1) * E2],
                                    s_sb[:, (j * 4 + k) * P:(j * 4 + k + 1) * P],
                                    b_sb[:, (j * 4 + k) * E2:(j * 4 + k + 1) * E2],
                                    start=(k == 0),
                                    stop=(k == 3),
                                )

                    if act_cp:
                        # double-read units -> fp16 sbuf via the scalar
                        # engine: halves the DVE's psum traffic, releases
                        # the psum banks one ACT-copy earlier, and makes
                        # those reduces all-16-bit (2x DVE mode eligible)
                        cp0 = pr_pool.tile([P, 1024], f16, name="cp0")
                        nc.scalar.copy(out=cp0[:], in_=us[0][:, 0:1024])
                        cp1 = pr_pool.tile([P, 1024], f16, name="cp1")
                        nc.scalar.copy(out=cp1[:], in_=us[1][:, 0:1024])
                        r0a, r0b = cp0[:, 0:1024], cp0[:, 0:1024]
                        r1a, r1b = cp1[:, 0:1024], cp1[:, 0:E2]
                    else:
                        r0a, r0b = us[0][:, 0:1024], us[0][:, 0:1024]
                        r1a, r1b = us[1][:, 0:1024], us[1][:, 0:E2]

                    def red(in0, g_lo, g_n, plane):
                        prod = pr_pool.tile([P, 1024], f32 if prod_f32 else f16,
                                            name="prod")
                        nc.vector.affine_mul_reduce(
                            out=prod[:, 0:g_n * E2],
                            accum_out=part[:, plane * G_PER + g:
                                           plane * G_PER + g + 1],
                            in0=in0,
                            in1=g_sb[:, g_lo * E2:(g_lo + g_n) * E2],
                            scale=1.0,
                            bias=0.0,
                        )

                    # out1 partials -> planes 0..2, out2 -> planes 3..5
                    red(r0a, 0, 2, 0)                # M3.g3 + M1.g1
                    red(r0b, 8, 2, 3)                # (M3+M1).g6
                    red(r1a, 6, 2, 1)                # (M4+M7).g1
                    red(r1b, 4, 1, 4)                # M4.g5
                    red(us[2][:, 0:1024], 2, 2, 5)   # M6.g6 + M2.g4
                    red(us[3][:, 0:E2], 5, 1, 2)     # M5.g2

                o1 = slice(rep * n_tiles, rep * n_tiles + G_PER)
                o2 = slice(rep * n_tiles + G_PER, rep * n_tiles + 2 * G_PER)
                pl = lambda j: part[:, j * G_PER:(j + 1) * G_PER]
                nc.vector.affine_then_add(
                    out=out_sb[:, o1], in0=pl(0), in1=pl(1), scale=1.0, bias=0.0)
                nc.vector.affine_then_add(
                    out=out_sb[:, o1], in0=out_sb[:, o1], in1=pl(2),
                    scale=1.0, bias=0.0)
                nc.vector.affine_then_add(
                    out=out_sb[:, o2], in0=pl(3), in1=pl(4), scale=1.0, bias=0.0)
                nc.vector.affine_then_add(
                    out=out_sb[:, o2], in0=out_sb[:, o2], in1=pl(5),
                    scale=1.0, bias=0.0)

            nc.sync.dma_start(out=out, in_=out_sb[:])

    nc.compile()
    return nc


def _make_runner(nc, n_cores):
    """Mirror bass2jax.run_bass_via_pjrt's multi-core branch, but return a
    cached jitted callable so repeat calls skip retracing."""
    import jax
    import concourse.mybir as mybir
    from concourse import bass2jax
    from jax.experimental.shard_map import shard_map
    from jax.sharding import Mesh, PartitionSpec

    bass2jax.install_neuronx_cc_hook()
    assert nc.dbg_addr is None
    partition_name = nc.partition_id_tensor.name if nc.partition_id_tensor else None

    in_names, out_names, out_avals = [], [], []
    for alloc in nc.m.functions[0].allocations:
        if not isinstance(alloc, mybir.MemoryLocationSet):
            continue
        name = alloc.memorylocations[0].name
        if alloc.kind == "ExternalInput":
            if name != partition_name:
                in_names.append(name)
        elif alloc.kind == "ExternalOutput":
            shape = tuple(alloc.tensor_shape)
            dtype = mybir.dt.np(alloc.dtype)
            out_names.append(name)
            out_avals.append(jax.core.ShapedArray(shape, dtype))
    n_params = len(in_names)
    n_outs = len(out_avals)
    all_in_names = in_names + out_names
    if partition_name is not None:
        all_in_names = all_in_names + [partition_name]

    def _body(*args):
        operands = list(args)
        if partition_name is not None:
            operands.append(bass2jax.partition_id_tensor())
        outs = bass2jax._bass_exec_p.bind(
            *operands,
            out_avals=tuple(out_avals),
            in_names=tuple(all_in_names),
            out_names=tuple(out_names),
            lowering_input_output_aliases=(),
            sim_require_finite=True,
            sim_require_nnan=True,
            nc=nc,
        )
        return tuple(outs)

    devices = jax.devices()[:n_cores]
    assert len(devices) == n_cores
    mesh = Mesh(np.asarray(devices), ("core",))
    spec = PartitionSpec("core")
    sharded = jax.jit(
        shard_map(
            _body,
            mesh=mesh,
            in_specs=(spec,) * (n_params + n_outs),
            out_specs=(spec,) * n_outs,
            check_rep=False,
        ),
        donate_argnums=tuple(range(n_params, n_params + n_outs)),
        keep_unused=True,
    )
    zero_out_shapes = [
        ((n_cores * av.shape[0],) + tuple(av.shape[1:]), av.dtype) for av in out_avals
    ]
    return sharded, in_names, out_names, zero_out_shapes, mesh, spec


def _get_runner():
    r = _CACHE.get("runner")
    if r is None:
        nc = _build_program_strassen(repeats=1)
        r = _CACHE["runner"] = _make_runner(nc, N_CORES)
    return r


def _strassen_inputs(elg, eth, weight):
    """Host marshalling: Strassen operand combinations, fp16, device layouts."""
    f32 = np.float32
    ec = elg.astype(f32).reshape(N_CORES, 2, H, 2, E2)
    A11, A12 = ec[:, 0, :, 0], ec[:, 0, :, 1]
    A21, A22 = ec[:, 1, :, 0], ec[:, 1, :, 1]
    S = [A11 + A22, A21 + A22, A11, A22, A11 + A12, A21 - A11, A12 - A22]

    W = weight.astype(f32)
    B11, B12 = W[:E2, :E2], W[:E2, E2:]
    B21, B22 = W[E2:, :E2], W[E2:, E2:]
    T = [B11 + B22, B11, B12 - B22, B21 - B11, B22, B11 + B12, B21 + B22]

    # sT16 [cores*16*P, 28*P]: [c, g, p, j*4+k, cc] = S[ORDER[j]][c, g*128+cc, k*128+p]
    Ss = np.stack([S[o] for o in ORDER], axis=1)          # [c, 7, H, 512]
    sT16 = np.ascontiguousarray(
        Ss.reshape(N_CORES, 7, G_PER, P, 4, P).transpose(0, 2, 5, 1, 4, 3)
    ).reshape(N_CORES * G_PER * P, 28 * P).astype(np.float16)

    # bT16 [cores*P, 28*512]: [p, j*4+k, e] = T[ORDER[j]][k*128+p, e]
    Ts = np.stack([T[o] for o in ORDER], axis=0)          # [7, 512, 512]
    bT = np.ascontiguousarray(
        Ts.reshape(7, 4, P, E2).transpose(2, 0, 1, 3)
    ).reshape(P, 28 * E2).astype(np.float16)
    bT16 = np.ascontiguousarray(
        np.broadcast_to(bT, (N_CORES,) + bT.shape)
    ).reshape(N_CORES * P, 28 * E2)

    # g16 [cores*16*P, 6*512], slot order (g3, g1, g6, g4, g5, g2)
    tc_ = eth.astype(f32).reshape(N_CORES, 2, H, 2, E2)
    w11, w12 = tc_[:, 0, :, 0], tc_[:, 0, :, 1]
    w21, w22 = tc_[:, 1, :, 0], tc_[:, 1, :, 1]
    Gs = np.stack([w12, w11, w22, w21 - w22, w21, w12 - w11], axis=2)
    g16 = np.ascontiguousarray(Gs).reshape(N_CORES * H, 6 * E2).astype(np.float16)

    return {"sT16": sT16, "g16": g16, "bT16": bT16}


def _call_runner(global_ins):
    sharded, in_names, out_names, zero_out_shapes, _, _ = _get_runner()
    zeros = [np.zeros(shape, dt) for shape, dt in zero_out_shapes]
    out_arrs = sharded(*[global_ins[n] for n in in_names], *zeros)
    out_g = np.asarray(out_arrs[out_names.index("out")])  # [8*128, 32]
    return np.concatenate(
        [out_g[c * P:(c + 1) * P].T.reshape(-1) for c in range(N_CORES)]
    ).astype(np.float32)


def kernel(elg, eth, weight):
    elg = np.asarray(elg, dtype=np.float32)
    eth = np.asarray(eth, dtype=np.float32)
    weight = np.asarray(weight, dtype=np.float32)
    return _call_runner(_strassen_inputs(elg, eth, weight))
